# revision 36
# baseline (speedup 1.0000x reference)
"""Trainium2 Bass kernel for a pre-norm transformer block with banded
(sliding-window) attention.

Reference computation (B=4, T=2048, D=512, H=8 heads, head_dim=64,
FFN=2048, fp32):
    xn = rmsnorm(x) ; qkv = xn @ w_qkv ; banded attention (|q-k| <= 64)
    x  = x + attn_out @ w_out + b_out
    h  = gelu(rmsnorm(x) @ w1 + b1) ; out = x + h @ w2 + b2

Sharding: token-parallel over 8 NeuronCores.  B*T = 8192 tokens -> 1024
tokens per core (each core takes half of one batch row).  Because the
attention is banded with context <= 64, each shard only needs a
64-token halo on each side; row edges are zero-padded and masked.  No
collectives.

Per-core design:
  - activations token-major [128 tokens, D] for norms/softmax (free-dim
    reductions), transposed to feature-major via PE transposes for use
    as matmul operands;
  - all matmuls bf16 with fp32 PSUM accumulation (fp32 matmul is 4x
    slower on the PE array); weights are cast to bf16 on the host and
    DMA'd directly;
  - the additive attention band mask is applied on the PE by
    accumulating identity.T @ mask into the scores PSUM;
  - softmax: exp on ScalarE straight from PSUM, row-sums + reciprocal
    on VectorE, normalize on GpSimd (free-dim broadcast multiply);
    heads processed in pairs so PSUM->SBUF copies move larger tiles;
  - rmsnorm: x^2 and its row-sum fused in one ScalarE op (Square +
    accum_out); normalize+gain fused in one VectorE scalar_tensor_tensor;
  - weight DMAs are emitted after the x-tile loads and the PE is warmed
    with dummy transposes so the first qkv matmuls hit the fast clock.
"""

import numpy as np
import copy as _copy

import concourse.bass as bass
import concourse.tile as tile
from concourse import mybir
from concourse.bass_utils import run_bass_kernel_spmd

F32 = mybir.dt.float32
BF16 = mybir.dt.bfloat16
FP8 = mybir.dt.float8e4
DR = mybir.MatmulPerfMode.DoubleRow
AX = mybir.AxisListType.X
AF = mybir.ActivationFunctionType
MUL = mybir.AluOpType.mult
PSUM = bass.MemorySpace.PSUM

B, T, D = 4, 2048, 512
H, HD = 8, 64
FFN = 2048
EPS = 1e-6
TPC = 1024          # tokens per core
HALO = 64
TPAD = TPC + 2 * HALO   # 1152 padded tokens
NB = TPAD // 128        # 9 padded token blocks
NQ = TPC // 128         # 8 query blocks


def _bcast_ap(src_1d, p=128):
    """DMA access pattern broadcasting a 1-D DRAM vector across p partitions."""
    return bass.AP(tensor=src_1d.tensor, offset=src_1d.offset,
                   ap=[[0, p]] + [list(a) for a in src_1d.ap])


def _split_waits(nc, maxw=1):
    """Stock walrus rejects instructions carrying more than `maxw` semaphore
    waits; move extras onto preceding no-ops on the same engine."""
    m = nc.m
    new_module = _copy.replace(m, functions=[])
    for function in m.functions:
        new_function = _copy.replace(function, blocks=[])
        new_function.set_allocations_from_list(function.allocations)
        for block in function.blocks:
            insts = []
            for inst in block.instructions:
                si = inst.sync_info
                if si is not None and len(si.on_wait) > maxw:
                    waits = list(si.on_wait)
                    extra, keep = waits[maxw:], waits[:maxw]
                    for j in range(0, len(extra), maxw):
                        insts.append(mybir.InstNoOp(
                            name=f"{inst.name}_wsplit{j}",
                            engine=inst.engine,
                            sync_info=mybir.SyncInfo(
                                on_wait=extra[j:j + maxw], on_update=[]),
                            bass_nofuse=True,
                        ))
                    inst.sync_info = mybir.SyncInfo(
                        on_wait=keep, on_update=list(si.on_update))
                insts.append(inst)
            new_function.blocks.append(_copy.replace(block, instructions=insts))
        new_module.functions.append(new_function)
    nc.m = new_module


def _build_nc():
    nc = bass.Bass("TRN2", debug=False)

    xpad_d = nc.dram_tensor("xpad", [TPAD, D], F32, kind="ExternalInput")
    masks_d = nc.dram_tensor("masks", [3, 128, 256], BF16, kind="ExternalInput")
    wqkv_d = nc.dram_tensor("w_qkv", [D, 3 * D], BF16, kind="ExternalInput")
    wout_d = nc.dram_tensor("w_out", [D, D], BF16, kind="ExternalInput")
    bout_d = nc.dram_tensor("b_out", [D], F32, kind="ExternalInput")
    w1_d = nc.dram_tensor("w1", [D, FFN], BF16, kind="ExternalInput")
    b1_d = nc.dram_tensor("b1", [FFN], F32, kind="ExternalInput")
    w2_d = nc.dram_tensor("w2", [FFN, D], BF16, kind="ExternalInput")
    b2_d = nc.dram_tensor("b2", [D], F32, kind="ExternalInput")
    n1_d = nc.dram_tensor("norm1_w", [D], F32, kind="ExternalInput")
    n2_d = nc.dram_tensor("norm2_w", [D], F32, kind="ExternalInput")
    id_d = nc.dram_tensor("ident", [128, 128], BF16, kind="ExternalInput")
    out_d = nc.dram_tensor("out", [TPC, D], F32, kind="ExternalOutput")

    with tile.TileContext(nc) as tc:
        with (
            tc.tile_pool(name="consts", bufs=1) as consts,
            tc.tile_pool(name="weights", bufs=1) as wpool,
            tc.tile_pool(name="acts", bufs=1) as acts,
            tc.tile_pool(name="xt", bufs=4) as xtp,
            tc.tile_pool(name="scr", bufs=4) as scr,
            tc.tile_pool(name="small", bufs=12) as small,
            tc.tile_pool(name="asb", bufs=10) as asb,
            tc.tile_pool(name="psum_mm", bufs=2, space=PSUM) as psum_mm,
            tc.tile_pool(name="psum_sc", bufs=3, space=PSUM) as psum_sc,
            tc.tile_pool(name="psum_tr", bufs=2, space=PSUM) as psum_tr,
            tc.tile_pool(name="psum_av", bufs=1, space=PSUM) as psum_av,
        ):
            # ---- constants -------------------------------------------------
            eps_t = consts.tile([128, 1], F32, tag="eps")
            nc.vector.memset(eps_t[:], EPS)
            zero_t = consts.tile([128, 1], F32, tag="zero")
            nc.vector.memset(zero_t[:], 0.0)
            warm2 = consts.tile([128, 1], F32, tag="warm")
            nc.scalar.activation(warm2[:], zero_t[:], AF.Square, bias=zero_t[:])
            x0 = xtp.tile([128, D], F32, tag="x")
            nc.sync.dma_start(x0[:], xpad_d[0:128, :])
            idt = consts.tile([128, 128], BF16)
            nc.sync.dma_start(idt[:], id_d[:])
            m_sb = consts.tile([128, 3, 256], BF16)
            nc.gpsimd.dma_start(m_sb[:], masks_d.rearrange("m p k -> p m k"))
            n1_b = consts.tile([128, D], F32, tag="n1")
            nc.gpsimd.dma_start(n1_b[:], _bcast_ap(n1_d[:]))
            n2_b = consts.tile([128, D], F32, tag="n2")
            nc.gpsimd.dma_start(n2_b[:], _bcast_ap(n2_d[:]))
            bout_b = consts.tile([128, D], F32, tag="bo")
            nc.gpsimd.dma_start(bout_b[:], _bcast_ap(bout_d[:]))
            b2_b = consts.tile([128, D], F32, tag="b2")
            nc.gpsimd.dma_start(b2_b[:], _bcast_ap(b2_d[:]))
            b1_fm = consts.tile([128, FFN // 128], F32, tag="b1")
            nc.gpsimd.dma_start(b1_fm[:], b1_d.rearrange("(m p) -> p m", p=128))

            # PE warm-up: dummy transposes keep the PE HAM activity window
            # alive while the first rmsnorm chain runs, so qkv starts at the
            # warm clock
            for wi in range(10):
                ptw = psum_tr.tile([128, 128], BF16, tag="tr", name=f"warmt{wi}")
                nc.tensor.transpose(ptw[:], idt[:], idt[:])

            # ---- weights (already bf16 from host) --------------------------
            # emitted lazily so the x-tile DMAs reach the queue first and
            # later weight loads overlap earlier compute phases
            def load_w(dram, kchunks, ncols, tag, dt=BF16):
                w = wpool.tile([128, kchunks, ncols], dt, tag=tag)
                for c in range(kchunks):
                    nc.sync.dma_start(w[:, c, :], dram[128 * c:128 * (c + 1), :])
                return w

            # ---- phase 1: load x, rmsnorm, transpose to feature-major ------
            def rmsnorm(xt, nw_b, xnb):
                """token-major rmsnorm: xnb = xt / rms(xt) * nw (bf16 out)"""
                s = scr.tile([128, D], F32, tag="s")
                sq = small.tile([128, 1], F32, tag="sq")
                nc.scalar.activation(s[:], xt[:], AF.Square, bias=zero_t[:],
                                     accum_out=sq[:])
                rms = small.tile([128, 1], F32, tag="rms")
                nc.scalar.activation(rms[:], sq[:], AF.Sqrt, bias=eps_t[:],
                                     scale=1.0 / D)
                inv = small.tile([128, 1], F32, tag="inv")
                nc.vector.reciprocal(inv[:], rms[:])
                nc.vector.scalar_tensor_tensor(xnb[:], xt[:], inv[:], nw_b[:],
                                               op0=MUL, op1=MUL)

            xnT = acts.tile([128, 4, TPAD], BF16, tag="xnT")
            for i in range(NB):
                if i == 0:
                    xt = x0
                else:
                    xt = xtp.tile([128, D], F32, tag="x")
                    eng = nc.sync if i % 2 == 0 else nc.gpsimd
                    eng.dma_start(xt[:], xpad_d[128 * i:128 * (i + 1), :])
                xnb = xtp.tile([128, D], BF16, tag="xnb")
                rmsnorm(xt, n1_b, xnb)
                for c in range(4):
                    pt = psum_tr.tile([128, 128], BF16, tag="tr")
                    nc.tensor.transpose(pt[:], xnb[:, 128 * c:128 * (c + 1)], idt[:])
                    if c % 2 == 0:
                        nc.vector.tensor_copy(xnT[:, c, 128 * i:128 * (i + 1)], pt[:])
                    else:
                        nc.scalar.copy(xnT[:, c, 128 * i:128 * (i + 1)], pt[:])

            wqkv = load_w(wqkv_d, 4, 3 * D, "wqkv")

            # ---- phase 2: qkv ---------------------------------------------
            # q, k feature-major [dim, tok]; v token-major [tok, dim]
            qf = acts.tile([128, 4, TPAD], BF16, tag="big")
            kf = acts.tile([128, 4, TPAD], BF16, tag="kf")
            for m in range(8):
                for j in range(3):
                    ps = psum_mm.tile([128, 384], F32, tag="mm")
                    for c in range(4):
                        nc.tensor.matmul(
                            ps[:], wqkv[:, c, 128 * m:128 * (m + 1)],
                            xnT[:, c, 384 * j:384 * (j + 1)],
                            start=(c == 0), stop=(c == 3))
                    if m < 4:  # q: fold in 1/sqrt(head_dim)
                        nc.scalar.activation(qf[:, m, 384 * j:384 * (j + 1)],
                                             ps[:], AF.Copy, scale=HD ** -0.5)
                    else:
                        nc.scalar.copy(kf[:, m - 4, 384 * j:384 * (j + 1)], ps[:])
            vt = acts.tile([128, NB, D], BF16, tag="vt")
            for i in range(NB):
                ps = psum_mm.tile([128, 512], F32, tag="mm")
                for c in range(4):
                    nc.tensor.matmul(ps[:], xnT[:, c, 128 * i:128 * (i + 1)],
                                     wqkv[:, c, 1024:1536],
                                     start=(c == 0), stop=(c == 3))
                nc.vector.tensor_copy(vt[:, i, :], ps[:])

            wout = load_w(wout_d, 4, D, "wout")
            w1b = load_w(w1_d, 4, FFN, "w1")
            w2b = load_w(w2_d, 16, D, "w2")

            # ---- phase 3: banded attention --------------------------------
            # Per head: scores = q.T k accumulated with an additive band mask
            # (identity.T @ mask on the PE); exp on ScalarE; row-sums +
            # reciprocal on VectorE; normalize on GpSimd (free-dim broadcast);
            # probability transposes on the PE.  Heads are processed in pairs
            # so PSUM->SBUF copies move [128,256]/[128,128] tiles.
            attn_f = acts.tile([128, 4, TPC], BF16, tag="a2")
            for qb in range(NQ):
                msel = 0 if qb == 0 else (2 if qb == NQ - 1 else 1)
                q0 = HALO + 128 * qb
                for hp in range(H // 2):
                    ch = hp
                    # both heads of the pair share one 512-wide scores bank:
                    # per-region scores+mask groups, then one exp / segmented
                    # row-sum / reciprocal for the pair
                    ps_s = psum_sc.tile([128, 512], F32, tag="sc")
                    for hi in range(2):
                        sl = slice(256 * hi, 256 * (hi + 1))
                        nc.tensor.matmul(
                            ps_s[:, sl],
                            qf[64 * hi:64 * hi + 64, ch, q0:q0 + 128],
                            kf[64 * hi:64 * hi + 64, ch,
                               128 * qb:128 * qb + 256],
                            start=True, stop=False)
                        nc.tensor.matmul(ps_s[:, sl], idt[:], m_sb[:, msel, :],
                                         start=False, stop=True)
                    pr = asb.tile([128, 512], BF16, tag="pr")
                    nc.scalar.activation(pr[:], ps_s[:], AF.Exp, bias=zero_t[:])
                    ssum = small.tile([128, 2], F32, tag="ssum")
                    nc.vector.reduce_sum(
                        ssum[:], pr[:].rearrange("p (a b) -> p a b", a=2),
                        axis=AX)
                    rcp = small.tile([128, 2], F32, tag="rcp")
                    nc.vector.reciprocal(rcp[:], ssum[:])
                    # all four of the pair's probability transposes land in
                    # one PSUM bank (own region/group each) -> single copy
                    ptr = psum_tr.tile([128, 4, 128], BF16, tag="tr",
                                       name=f"ptr{qb}_{hp}")
                    for hi in range(2):
                        prn = asb.tile([128, 256], BF16, tag="prn",
                                       name=f"prn{qb}_{hp}_{hi}")
                        nc.gpsimd.tensor_tensor(
                            prn[:], pr[:, 256 * hi:256 * (hi + 1)],
                            rcp[:, hi:hi + 1].to_broadcast([128, 256]), MUL)
                        for w in range(2):
                            nc.tensor.transpose(
                                ptr[:, 2 * hi + w, :],
                                prn[:, 128 * w:128 * (w + 1)], idt[:])
                    pT4 = asb.tile([128, 4, 128], BF16, tag="pT",
                                   name=f"pT{qb}_{hp}")
                    if hp % 2 == 0:
                        nc.vector.tensor_copy(pT4[:], ptr[:])
                    else:
                        nc.scalar.copy(pT4[:], ptr[:])
                    ps_av = psum_av.tile([128, 128], F32, tag="av")
                    for hi in range(2):
                        h = 2 * hp + hi
                        for w in range(2):
                            nc.tensor.matmul(
                                ps_av[64 * hi:64 * hi + 64, :],
                                vt[:, qb + w, 64 * h:64 * (h + 1)],
                                pT4[:, 2 * hi + w, :],
                                start=(w == 0), stop=(w == 1))
                    if hp % 2 == 0:
                        nc.vector.tensor_copy(
                            attn_f[:, ch, 128 * qb:128 * (qb + 1)], ps_av[:])
                    else:
                        nc.scalar.copy(
                            attn_f[:, ch, 128 * qb:128 * (qb + 1)], ps_av[:])

            # ---- phase 4: out-proj + residual + rmsnorm2 -------------------
            x2_all = acts.tile([128, NQ, D], F32, tag="x2")
            xn2T = acts.tile([128, 4, TPC], BF16, tag="xn2T")
            for j in range(NQ):
                ps = psum_mm.tile([128, 512], F32, tag="mm")
                for c in range(4):
                    nc.tensor.matmul(ps[:], attn_f[:, c, 128 * j:128 * (j + 1)],
                                     wout[:, c, :], start=(c == 0), stop=(c == 3))
                xc = xtp.tile([128, D], F32, tag="x")
                nc.sync.dma_start(xc[:], xpad_d[HALO + 128 * j:HALO + 128 * (j + 1), :])
                t1 = scr.tile([128, D], F32, tag="s")
                nc.vector.tensor_add(t1[:], ps[:], bout_b[:])
                x2 = x2_all[:, j, :]
                nc.vector.tensor_add(x2, t1[:], xc[:])
                xn2b = xtp.tile([128, D], BF16, tag="xnb")
                rmsnorm(x2, n2_b, xn2b)
                for c in range(4):
                    pt = psum_tr.tile([128, 128], BF16, tag="tr")
                    nc.tensor.transpose(pt[:], xn2b[:, 128 * c:128 * (c + 1)], idt[:])
                    nc.vector.tensor_copy(xn2T[:, c, 128 * j:128 * (j + 1)], pt[:])

            # ---- phase 5: FFN ---------------------------------------------
            hf = acts.tile([128, 16, TPC], BF16, tag="big")
            for m in range(16):
                ps0 = psum_mm.tile([128, 512], F32, tag="mm")
                ps1 = psum_mm.tile([128, 512], F32, tag="mm")
                for c in range(4):  # shared stationary weight for both halves
                    nc.tensor.matmul(ps0[:], w1b[:, c, 128 * m:128 * (m + 1)],
                                     xn2T[:, c, 0:512],
                                     start=(c == 0), stop=(c == 3))
                    nc.tensor.matmul(ps1[:], w1b[:, c, 128 * m:128 * (m + 1)],
                                     xn2T[:, c, 512:1024],
                                     start=(c == 0), stop=(c == 3))
                nc.scalar.activation(hf[:, m, 0:512], ps0[:], AF.Gelu,
                                     bias=b1_fm[:, m:m + 1], scale=1.0)
                nc.scalar.activation(hf[:, m, 512:1024], ps1[:], AF.Gelu,
                                     bias=b1_fm[:, m:m + 1], scale=1.0)
            for j in range(NQ):
                ps = psum_mm.tile([128, 512], F32, tag="mm")
                for m in range(16):
                    nc.tensor.matmul(ps[:], hf[:, m, 128 * j:128 * (j + 1)],
                                     w2b[:, m, :],
                                     start=(m == 0), stop=(m == 15))
                o1 = scr.tile([128, D], F32, tag="s")
                nc.vector.tensor_add(o1[:], ps[:], b2_b[:])
                o2 = xtp.tile([128, D], F32, tag="o2")
                nc.vector.tensor_add(o2[:], o1[:], x2_all[:, j, :])
                nc.sync.dma_start(out_d[128 * j:128 * (j + 1), :], o2[:])

    nc.finalize()
    _split_waits(nc)
    return nc


_NC = None


def _get_nc():
    global _NC
    if _NC is None:
        _NC = _build_nc()
    return _NC


def _make_in_maps(x, norm1_w, norm2_w, w_qkv, w_out, b_out, w1, b1, w2, b2,
                  context_size):
    import ml_dtypes
    bf16 = ml_dtypes.bfloat16
    c = int(np.asarray(context_size))
    assert c <= HALO, f"context_size {c} exceeds compiled halo {HALO}"
    x = np.ascontiguousarray(np.asarray(x, np.float32))
    shared = {
        "w_qkv": np.ascontiguousarray(np.asarray(w_qkv).astype(bf16)),
        "w_out": np.ascontiguousarray(np.asarray(w_out).astype(bf16)),
        "w1": np.ascontiguousarray(np.asarray(w1).astype(bf16)),
        "w2": np.ascontiguousarray(np.asarray(w2).astype(bf16)),
        "b_out": np.ascontiguousarray(np.asarray(b_out, np.float32)),
        "b1": np.ascontiguousarray(np.asarray(b1, np.float32)),
        "b2": np.ascontiguousarray(np.asarray(b2, np.float32)),
        "norm1_w": np.ascontiguousarray(np.asarray(norm1_w, np.float32)),
        "norm2_w": np.ascontiguousarray(np.asarray(norm2_w, np.float32)),
        "ident": np.eye(128, dtype=bf16),
    }
    in_maps = []
    i = np.arange(128)[:, None]
    jj = np.arange(256)[None, :]
    for core in range(8):
        b, t0 = core // 2, (core % 2) * TPC
        lo, hi = t0 - HALO, t0 + TPC + HALO
        xp = np.zeros((TPAD, D), np.float32)
        s0, s1 = max(lo, 0), min(hi, T)
        xp[s0 - lo:s0 - lo + (s1 - s0)] = x[b, s0:s1]
        masks = np.empty((3, 128, 256), np.float32)
        for mi, qb in ((0, 0), (1, 3), (2, NQ - 1)):
            qg = t0 + 128 * qb + i
            kg = t0 - HALO + 128 * qb + jj
            ok = (np.abs(qg - kg) <= c) & (kg >= 0) & (kg < T)
            masks[mi] = np.where(ok, np.float32(0.0), np.float32(-30000.0))
        in_maps.append({"xpad": xp, "masks": masks.astype(bf16), **shared})
    return in_maps


def _run(in_maps, **kwargs):
    return run_bass_kernel_spmd(_get_nc(), in_maps, core_ids=list(range(8)),
                                **kwargs)


def kernel(**inputs):
    in_maps = _make_in_maps(**inputs)
    res = _run(in_maps)
    out = np.empty((B, T, D), np.float32)
    for core in range(8):
        b, t0 = core // 2, (core % 2) * TPC
        out[b, t0:t0 + TPC] = res.results[core]["out"]
    return out
